# revision 23
# baseline (speedup 1.0000x reference)
"""Trainium2 Bass kernel for CPELayer_ResAG (concept-routed LoRA edit layer).

Computation (per token t with concept c = concept_idx[t]):
    down = edit_direction[t] @ lora_down[c]          # [768]@[768,4] -> [4]
    up   = down @ lora_up[c]                         # [4]@[4,1280]  -> [1280]
    out  = x[t] @ W.T + b_lin + 0.25 * up

Strategy: data-parallel over batch across 8 cores (616 tokens/core).
The routed LoRA is computed densely: A.T[(c,r), t] = lora_down_flat.T @ ed.T
for ALL concepts (only ~6% extra PE work), then masked on-device with a
one-hot built by DVE is_equal (the MoE routing), and contracted back with
lora_up_flat via the tensor engine, accumulating into the same PSUM as the
org matmul.  The bias is folded in as one extra contraction row (ones row in
the masked operand, b_lin row in the lora_up operand).  The 0.25 LoRA scale
is folded into lora_up host-side (exact: power of two).

All host-side work is layout only (transpose / reshape / concat / dtype of
the int indices to f32); every FLOP of the reference runs on device.
"""

import sys
import types

import numpy as np

import concourse.mybir as mybir
import concourse.tile as tile
from concourse import bacc
from concourse.bass_utils import run_bass_kernel_spmd

# If BASS_TRACE is set in the environment, run_bass_kernel_spmd imports
# antenv.axon_hooks, which some containers lack; stub it (None hook ->
# tracing is skipped gracefully, execution unaffected).
try:
    import antenv.axon_hooks  # noqa: F401
except ImportError:
    _m = types.ModuleType("antenv.axon_hooks")
    _m.get_axon_ntff_profile_hook = lambda: None
    _m.set_axon_ntff_profile_hook = lambda h: None
    sys.modules["antenv.axon_hooks"] = _m

# Problem shapes (hardcoded per spec nn_CPELayer_ResAG_19335942766951)
N_CORES = 8
B, T, DIN, DOUT = 64, 77, 768, 1280
N_CONCEPTS, RANK = 50, 4
SCALE = 0.25  # alpha/rank = 1/4, exact power of two
BPC = B // N_CORES          # batches per core = 8
TOK = BPC * T               # tokens per core = 616
NJ = N_CONCEPTS * RANK      # 200 flattened (concept, rank) rows
KJ_PAD = 256                # padded rows: 200 lora + 1 bias + 55 zero
P = 128
KD = DIN // P               # 6 k-tiles of the d_in contraction
NH = 308                    # half of TOK for the A.T psum tiles (>=256 keeps f32r full-rate)
T_EDGES = [0, 128, 256, 384, 512, 616]
N_CHUNKS = [(0, 512), (512, 512), (1024, 256)]

_cache = {}


def _build_bass(mm_dtype):
    nc = bacc.Bacc("TRN2", target_bir_lowering=False, debug=False,
                   num_devices=N_CORES)
    f32 = mybir.dt.float32
    # Tensors consumed by the tensor engine carry the matmul dtype end-to-end
    # (float32r is fp32-layout; the BIR verifier requires producer outputs to
    # be fp32r-typed when a fp32r matmul consumes them).
    sdt = mm_dtype

    xT_d = nc.dram_tensor("xT", [DIN, TOK], sdt, kind="ExternalInput").ap()
    edT_d = nc.dram_tensor("edT", [DIN, TOK], sdt, kind="ExternalInput").ap()
    idx_d = nc.dram_tensor("idxf", [1, TOK], f32, kind="ExternalInput").ap()
    cv_d = nc.dram_tensor("cvals", [P, 2], f32, kind="ExternalInput").ap()
    WT_d = nc.dram_tensor("WT", [DIN, DOUT], sdt, kind="ExternalInput").ap()
    ldT_d = nc.dram_tensor("ldT", [DIN, NJ], sdt, kind="ExternalInput").ap()
    lu_d = nc.dram_tensor("luB", [KJ_PAD, DOUT], sdt, kind="ExternalInput").ap()
    out_d = nc.dram_tensor("out", [TOK, DOUT], f32, kind="ExternalOutput").ap()

    with tile.TileContext(nc) as tc:
        with (
            tc.tile_pool(name="consts", bufs=1) as consts,
            tc.tile_pool(name="outsb", bufs=5) as outsb,
        ):
            # Load order matters: tiny routing tensors first (masks unblock),
            # then ldT/edT (the A.T chain), then luB (up-matmul rhs) so the
            # early wave-A matmuls can run, and the bulky org operands
            # (xT/WT) last, streaming k-pair by k-pair with the org matmuls
            # tracking their arrival.
            cvals = consts.tile([P, 2], f32, tag="cvals")
            nc.sync.dma_start(cvals[:], cv_d[:, :])

            # Broadcast the token->concept ids across all 128 partitions so a
            # per-partition-scalar is_equal against cvals builds the one-hot.
            idx_bc = consts.tile([P, TOK], f32, tag="idx_bc")
            nc.sync.dma_start(idx_bc[:], idx_d.partition_broadcast(P))

            xT = [None] * KD
            WT = [None] * KD

            def load_kpair(k):
                t_ = consts.tile([P, TOK], sdt, tag=f"xT{k}")
                nc.sync.dma_start(t_[:], xT_d[k * P:(k + 1) * P, :])
                xT[k] = t_
                t_ = consts.tile([P, DOUT], sdt, tag=f"WT{k}")
                nc.sync.dma_start(t_[:], WT_d[k * P:(k + 1) * P, :])
                WT[k] = t_

            ldT = []
            edT = []
            for k in range(KD):
                t_ = consts.tile([P, NJ], sdt, tag=f"ldT{k}")
                nc.sync.dma_start(t_[:], ldT_d[k * P:(k + 1) * P, :])
                ldT.append(t_)
                t_ = consts.tile([P, TOK], sdt, tag=f"edT{k}")
                nc.sync.dma_start(t_[:], edT_d[k * P:(k + 1) * P, :])
                edT.append(t_)
            lu = []
            for j in range(2):
                t_ = consts.tile([P, DOUT], sdt, tag=f"lu{j}")
                nc.sync.dma_start(t_[:], lu_d[j * P:(j + 1) * P, :])
                lu.append(t_)
            for k in range(KD):
                load_kpair(k)

            masks = []
            for jc in range(2):
                m = consts.tile([P, TOK], f32, tag=f"mask{jc}")
                nc.vector.tensor_scalar(
                    m[:], idx_bc[:], cvals[:, jc:jc + 1], None,
                    mybir.AluOpType.is_equal)
                masks.append(m)

            # A.T[(c,r), t] = lora_down_flat.T @ ed.T  for all concepts,
            # masked into MT (the routed "down" activations, transposed).
            MT = []
            for jc in range(2):
                t_ = consts.tile([P, TOK], sdt, tag=f"MT{jc}")
                MT.append(t_)
            # Chunk-1 rows 72..127 pair with luB rows 200..255: engine ops
            # need a 32-aligned start partition, so zero 64..128 first, then
            # the ones row at 96 (bias: b_lin sits at luB[224]); the mask-mul
            # below overwrites rows 0..71 (lora j=128..199).
            # (memset can't target float32r; synthesize 0s/1s via DVE with
            # idx_bc as a donor input, converted on write)
            nc.vector.tensor_scalar(
                MT[1][64:P, :], idx_bc[64:P, :], 0.0, None,
                mybir.AluOpType.mult)
            nc.vector.tensor_scalar(
                MT[1][96:97, :], idx_bc[96:97, :], 0.0, 1.0,
                mybir.AluOpType.mult, mybir.AluOpType.add)

            with tc.tile_pool(name="at_ps", bufs=4, space="PSUM") as at_pool:
                for jc in range(2):
                    jp = P if jc == 0 else NJ - P  # 128, 72
                    jsl = slice(jc * P, jc * P + jp)
                    for nh in range(2):
                        nsl = slice(nh * NH, (nh + 1) * NH)
                        at = at_pool.tile([P, NH], f32, tag="at")
                        for k in range(KD):
                            nc.tensor.matmul(
                                at[:jp, :], ldT[k][:, jsl], edT[k][:, nsl],
                                start=(k == 0), stop=(k == KD - 1))
                        nc.vector.tensor_tensor(
                            MT[jc][:jp, nsl], at[:jp, :], masks[jc][:jp, nsl],
                            mybir.AluOpType.mult)

            # Main accumulation, two short-lived PSUM waves per (t, n) so
            # banks recycle during the load phase instead of every group
            # staying open until the last WT k-tile arrives:
            #   wave A: up1+up2 (MT/lu ready early) + org k0..k2 -> copy osb
            #   wave B: org k3..k5 -> DVE-add into osb
            KA = 3  # org k-tiles in wave A
            with tc.tile_pool(name="out_ps", bufs=8, space="PSUM") as out_pool:
                osbs = []
                for ti in range(len(T_EDGES) - 1):
                    t0, t1 = T_EDGES[ti], T_EDGES[ti + 1]
                    tw = t1 - t0
                    tsl = slice(t0, t1)
                    osb = outsb.tile([P, DOUT], f32, tag="osb")
                    osbs.append(osb)
                    for (n0, nw) in N_CHUNKS:
                        ps = out_pool.tile([P, 512], f32, tag="ops")
                        nmm = 2 + KA
                        i = 0
                        for jc in range(2):
                            nc.tensor.matmul(
                                ps[:tw, :nw], MT[jc][:, tsl],
                                lu[jc][:, n0:n0 + nw],
                                start=(i == 0), stop=(i == nmm - 1))
                            i += 1
                        for k in range(KA):
                            nc.tensor.matmul(
                                ps[:tw, :nw], xT[k][:, tsl],
                                WT[k][:, n0:n0 + nw],
                                start=(i == 0), stop=(i == nmm - 1))
                            i += 1
                        nc.any.tensor_copy(out=osb[:tw, n0:n0 + nw],
                                           in_=ps[:tw, :nw])
                for ti in range(len(T_EDGES) - 1):
                    t0, t1 = T_EDGES[ti], T_EDGES[ti + 1]
                    tw = t1 - t0
                    tsl = slice(t0, t1)
                    osb = osbs[ti]
                    for (n0, nw) in N_CHUNKS:
                        ps = out_pool.tile([P, 512], f32, tag="ops")
                        for i, k in enumerate(range(KA, KD)):
                            nc.tensor.matmul(
                                ps[:tw, :nw], xT[k][:, tsl],
                                WT[k][:, n0:n0 + nw],
                                start=(i == 0), stop=(i == KD - KA - 1))
                        nc.vector.tensor_tensor(
                            osb[:tw, n0:n0 + nw], ps[:tw, :nw],
                            osb[:tw, n0:n0 + nw], mybir.AluOpType.add)
                    nc.sync.dma_start(out_d[tsl, :], osb[:tw, :])

    nc.compile()
    return nc


def get_bass(mm_dtype=None):
    if mm_dtype is None:
        mm_dtype = mybir.dt.float32r
    key = str(mm_dtype)
    if key not in _cache:
        _cache[key] = _build_bass(mm_dtype)
    return _cache[key]


def make_in_maps(x, edit_direction, concept_idx, lora_down, lora_up, W, b_lin,
                 np_sdt=np.float32):
    """Host-side sharding + layout prep (no reference FLOPs).

    np_sdt: numpy dtype for the matmul-side tensors (np.float32 for
    float32/float32r programs, ml_dtypes.bfloat16 for bf16)."""
    x = np.asarray(x, dtype=np.float32)
    ed = np.asarray(edit_direction, dtype=np.float32)
    idx = np.asarray(concept_idx)
    ld = np.asarray(lora_down, dtype=np.float32)
    lup = np.asarray(lora_up, dtype=np.float32)
    W = np.asarray(W, dtype=np.float32)
    b = np.asarray(b_lin, dtype=np.float32)

    WT = np.ascontiguousarray(W.T.astype(np_sdt))               # [768, 1280]
    ldT = np.ascontiguousarray(
        ld.transpose(1, 0, 2).reshape(DIN, NJ).astype(np_sdt))
    luB = np.zeros((KJ_PAD, DOUT), dtype=np.float32)
    luB[:NJ] = lup.reshape(NJ, DOUT) * SCALE                    # exact x0.25
    luB[128 + 96] = b                                           # bias row
    luB = luB.astype(np_sdt)
    cv = np.full(2 * P, -1.0, dtype=np.float32)
    cv[:NJ] = np.arange(NJ, dtype=np.float32) // RANK
    cvals = np.ascontiguousarray(cv.reshape(2, P).T)            # [128, 2]

    in_maps = []
    for c in range(N_CORES):
        sl = slice(c * BPC, (c + 1) * BPC)
        xs = x[sl].reshape(TOK, DIN)
        eds = ed[sl].reshape(TOK, DIN)
        idxs = idx[sl].reshape(TOK).astype(np.float32)
        in_maps.append({
            "xT": np.ascontiguousarray(xs.T.astype(np_sdt)),
            "edT": np.ascontiguousarray(eds.T.astype(np_sdt)),
            "idxf": np.ascontiguousarray(idxs.reshape(1, TOK)),
            "cvals": cvals,
            "WT": WT,
            "ldT": ldT,
            "luB": luB,
        })
    return in_maps


def kernel(x, edit_direction, concept_idx, lora_down, lora_up, W, b_lin,
           _trace=False, _mm_dtype=None):
    nc = get_bass(_mm_dtype)
    np_sdt = mybir.dt.np(_mm_dtype) if _mm_dtype is not None else np.float32
    in_maps = make_in_maps(x, edit_direction, concept_idx, lora_down, lora_up,
                           W, b_lin, np_sdt=np_sdt)
    res = run_bass_kernel_spmd(nc, in_maps, core_ids=list(range(N_CORES)),
                               trace=_trace)
    out = np.concatenate([r["out"] for r in res.results], axis=0)
    out = out.reshape(B, T, DOUT)
    if _trace:
        kernel.last_results = res
    return out
